# revision 43
# baseline (speedup 1.0000x reference)
# Cross-modal contrastive loss (forward) on 8 Trainium2 NeuronCores.
#
# Reference computation:
#   rgb2d = l2norm over C of rgb (B,C,H,W) -> (N=B*H*W, C)
#   x2d   = l2norm over C of x
#   sim   = rgb2d @ x2d.T / T                     (N x N, N = 8192)
#   mask[m, n] = (m // 1024 == n % 8)             (1024 positives per row)
#   loss = -(sum_pos (sim - logsumexp_row)) / (N*1024 + 1e-8)
#
# Sharding: core d owns rgb batch d (rows m in [1024 d, 1024 d + 1024)) and
# all of x.  Each core returns per-partition partials of
#   L = sum_m log(sum_n exp(sim[m, n]))  and  P = sum_m sum_{pos} sim[m,n]*rs/T
# and the host combines:  loss = -(P_tot - 1024 * L_tot) / (N*1024 + 1e-8).
#
# Key structure (v2):
#  - Host permutes x columns to n' = (n%8)*1024 + n//8 so each core's
#    positive columns form one contiguous 1024-wide slab (row sums of the
#    logsumexp are permutation-invariant).  Host also pre-casts inputs to
#    bf16/fp8 (marshalling only; all math runs on device).
#  - The N^2/8 exp+row-sum drain is the bottleneck; PSUM is drainable only
#    by ACT and DVE, so the 32 (row-block x column-group) tiles split:
#      A-tiles: ACT exp, per-row sums fused via accum_out (in-place PSUM).
#      V-tiles: DVE Schraudolph bits = int16(raw*rs*A16/T + B16) from PSUM;
#               the int16 bitcast as bf16 IS ~exp(s); an in-place bf16
#               tensor_scalar (4x DVE mode) + accum_out sums each row.
#  - Matmuls: groups 0-1 use plain bf16 (no cast latency); groups 2-3 use
#    fp8e4 DoubleRow (2 k-planes/instr, 4x bf16 throughput) from
#    on-device-normalized, fp8-cast x.
#  - Column norms are computed per 1024-col slab in a transposed [128, 8]
#    layout (ones-matmul per 128-col chunk) so the Ln/Exp ACT work is 8
#    wide; a transposing DRAM store + partition-broadcast load distributes
#    16/||col|| to all partitions.  Slab 0 instead computes its norms in
#    broadcast layout directly on the then-idle ACT engine (no DMA round
#    trip), so the first drain starts ~7us in.  The main loop runs in
#    half-sweeps (all row-blocks of slab 2g, then slab 2g+1), halving the
#    supply latency each tile waits on.
#  - Pool (gpsimd) cannot touch PSUM or run reduce ops, so it absorbs x^2
#    elementwise chunks (eff 0.42) plus the fp8 cast DMAs; the sync-ring
#    DMA order is hand-sequenced (loads split into 1024-col pieces so the
#    tiny inv round-trips interleave with them on the serialized DMA
#    engine).

import os

import numpy as np
import ml_dtypes

import concourse.bass as bass
import concourse.tile as tile
from concourse import bacc
from concourse import mybir
from concourse.bass_utils import run_bass_kernel_spmd

F32 = mybir.dt.float32
BF16 = mybir.dt.bfloat16
FP8 = mybir.dt.float8e4
I16 = mybir.dt.int16
AF = mybir.ActivationFunctionType
ALU = mybir.AluOpType

B, C, HW = 8, 256, 1024
N = B * HW            # 8192 total rows/cols of sim
KB = C // 128         # 2 contraction blocks
MB = HW // 128        # 8 m-blocks per core
GW = 2048             # column-group width
NG = N // GW          # 4 column groups
TEMP = 0.1
N_CORES = 8

# Schraudolph int16/bf16 exp: bits = round(s*A16 + B16), bitcast bf16 ~ exp(s)
A16 = 128.0 / float(np.log(2.0))
C16 = 7.35            # calibrated: zero mean bias of sum(exp) for s~N(0,.65)
B16 = 127.0 * 128.0 - C16
FQ = 16.0             # fp8 range scaling folded into the x normalization

V_SET = {2, 6, 10, 13, 16, 18, 20, 23, 25, 27, 29}
POOL_SQ = {(2, 1), (3, 1), (4, 1), (5, 1), (6, 0), (6, 1),
           (7, 0), (7, 1)}           # x^2 (slab, k) chunks on Pool
POOL_AP = set()                      # norm-apply (slab, k) chunks on Pool
BF16_GROUPS = {0, 1}                   # groups whose matmuls use bf16 xn

_CACHE = {}
LAST_RESULT = None    # BassKernelResults of the most recent run (for tests)


class _OneTableBacc(bacc.Bacc):
    """Resolve Exp/Ln/Square/Copy to the single natural_log_exp_and_others
    table set so the kernel needs exactly one ACT_TABLE_LOAD."""

    def insert_act_table_loads(self):
        from concourse.bacc import get_activation_tables
        import bass_rust as _bass_rust

        has = any(
            isinstance(i, mybir.InstActivation)
            for b in self.main_func.blocks
            for i in b.instructions
        )
        if not has:
            return
        tables = list(get_activation_tables(self.m.arch).items())
        out = []
        for idx, (name, fns) in enumerate(tables):
            if idx < 6 and name != "natural_log_exp_and_others":
                out.append((name, type(fns)()))
            else:
                out.append((name, fns))
        _bass_rust.insert_act_table_loads(self, out)


def _build_nc():
    nc = _OneTableBacc()
    x_h = nc.dram_tensor("xr", [KB, 128, N], BF16, kind="ExternalInput")
    rgb_h = nc.dram_tensor("rgbr", [KB, 128, HW], BF16, kind="ExternalInput")
    rgbq_h = nc.dram_tensor("rgbq", [KB, 128, HW], FP8, kind="ExternalInput")
    sel_h = nc.dram_tensor("selv", [8], BF16, kind="ExternalInput")
    scr_h = [nc.dram_tensor(f"scr{g}", [GW], BF16, kind="Internal")
             for g in range(NG)]
    out_h = nc.dram_tensor("out", [128, 2], F32, kind="ExternalOutput")

    with tile.TileContext(nc) as tc:
        with (
            tc.tile_pool(name="persist", bufs=1) as persist,
            tc.tile_pool(name="x2", bufs=3) as x2p,
            tc.tile_pool(name="invb", bufs=3) as invp,
            tc.tile_pool(name="bits", bufs=6) as bitsp,
            tc.tile_pool(name="small", bufs=1) as small,
            tc.tile_pool(name="mm", bufs=3, space="PSUM") as mmp,
            tc.tile_pool(name="sm", bufs=2, space="PSUM") as smp,
        ):
            ones_b = persist.tile([128, 128], BF16)
            nc.vector.memset(ones_b, 1.0)

            xr = persist.tile([128, KB, N], BF16, name="xr")
            xq = persist.tile([128, KB, N], FP8, name="xq")
            rgbr = persist.tile([128, KB, HW], BF16, name="rgbr")
            rgbq = persist.tile([128, KB, HW], FP8, name="rgbq")

            accums = small.tile([128, 64], F32)
            nc.vector.memset(accums, 0.0)
            pslab = small.tile([128, KB, 8], F32)
            out_sb = small.tile([128, 2], F32)
            se = small.tile([128, MB], F32)
            logs = small.tile([128, MB], F32)

            # ---- x(g0) first (heads the longest chain), then rgb; later
            #      x loads are split into 1024-col pieces so the serialized
            #      DMA engine interleaves the latency-critical inv
            #      store/broadcast transfers between them ----
            def load_x(g, piece=GW):
                for c0 in range(g * GW, (g + 1) * GW, piece):
                    cols = slice(c0, c0 + piece)
                    for k in range(KB):
                        nc.sync.dma_start(out=xr[:, k, cols],
                                          in_=x_h[k, :, cols])

            load_x(0, piece=1024)
            nc.sync.dma_start(out=rgbr, in_=rgb_h[:, :, :].rearrange("k c h -> c k h"))

            # ---- slab 0a (cols 0..1024) norms computed in BROADCAST layout
            #      entirely on the (idle) ACT engine: no DMA round-trip, so
            #      the first main-loop drain starts as early as possible ----
            x2a = x2p.tile([128, KB, 1024], BF16, tag="x2", name="x2_0a")
            for k in range(KB):
                nc.vector.tensor_mul(out=x2a[:, k, :], in0=xr[:, k, 0:1024],
                                     in1=xr[:, k, 0:1024])
            ss_bc = mmp.tile([128, 1024], F32, tag="mm", name="ss_bc")
            for tt in range(2):
                for k in range(KB):
                    nc.tensor.matmul(
                        ss_bc[:, tt * 512:(tt + 1) * 512],
                        lhsT=ones_b,
                        rhs=x2a[:, k, tt * 512:(tt + 1) * 512],
                        start=(k == 0), stop=(k == KB - 1),
                    )
            lnb = small.tile([128, 1024], F32)
            nc.scalar.activation(out=lnb, in_=ss_bc, func=AF.Ln,
                                 scale=1.0 / 256.0)
            iva = invp.tile([128, 1024], BF16, tag="invb", name="invb0a")
            nc.scalar.activation(out=iva, in_=lnb, func=AF.Exp, scale=-0.5)
            for k in range(KB):
                nc.vector.tensor_mul(out=xr[:, k, 0:1024],
                                     in0=xr[:, k, 0:1024], in1=iva)

            r2 = small.tile([128, KB, HW], BF16)
            for k in range(KB):
                nc.vector.tensor_mul(out=r2[:, k, :], in0=rgbr[:, k, :],
                                     in1=rgbr[:, k, :])
            ssr_ps = smp.tile([128, MB], F32, tag="sm")
            for j in range(MB):
                for k in range(KB):
                    nc.tensor.matmul(
                        ssr_ps[:, j:j + 1],
                        lhsT=r2[:, k, j * 128:(j + 1) * 128],
                        rhs=ones_b[:, 0:1],
                        start=(k == 0), stop=(k == KB - 1),
                    )
            lssr = small.tile([128, MB], F32)
            nc.scalar.activation(out=lssr, in_=ssr_ps, func=AF.Ln)
            rs = small.tile([128, MB], F32)
            nc.scalar.activation(out=rs, in_=lssr, func=AF.Exp, scale=-0.5)
            # scale_a = rs/(FQ*T) (ACT exp scale + positives), scale_v adds A16
            scale_a = small.tile([128, MB], F32)
            nc.vector.tensor_scalar_mul(out=scale_a, in0=rs,
                                        scalar1=1.0 / (FQ * TEMP))
            scale_v = small.tile([128, MB], F32)
            nc.vector.tensor_scalar_mul(out=scale_v, in0=rs,
                                        scalar1=A16 / (FQ * TEMP))

            # ---- per-slab (1024-col) x column norms + apply + positives
            #      slab sum; transposed layout, distributed via a
            #      transposing DRAM store + partition-broadcast load ----
            def norm_slab(s):
                g, h = s // 2, s % 2
                c0, w = s * 1024, 1024
                cols = slice(c0, c0 + w)
                x2 = x2p.tile([128, KB, w], BF16, tag="x2", name=f"x2_{s}")
                for k in range(KB):
                    eng = nc.gpsimd if (s, k) in POOL_SQ else nc.vector
                    eng.tensor_mul(out=x2[:, k, :], in0=xr[:, k, cols],
                                   in1=xr[:, k, cols])
                ss_t = smp.tile([128, w // 128], F32, tag="sm",
                                name=f"ss_t{s}")
                for c in range(w // 128):
                    for k in range(KB):
                        nc.tensor.matmul(
                            ss_t[:, c:c + 1],
                            lhsT=x2[:, k, c * 128:(c + 1) * 128],
                            rhs=ones_b[:, 0:1],
                            start=(k == 0), stop=(k == KB - 1),
                        )
                # inv16 = FQ/sqrt(ss): exp(-0.5*ln(ss*2^-8)) = 16/sqrt(ss)
                lnt = small.tile([128, w // 128], F32, tag=f"lnt{s}",
                                 name=f"lnt{s}")
                nc.scalar.activation(out=lnt, in_=ss_t, func=AF.Ln,
                                     scale=1.0 / 256.0)
                invt = small.tile([128, w // 128], BF16, tag=f"invt{s}",
                                  name=f"invt{s}")
                nc.scalar.activation(out=invt, in_=lnt, func=AF.Exp,
                                     scale=-0.5)
                o0 = h * 1024
                nc.sync.dma_start(
                    out=scr_h[g][o0:o0 + w].rearrange("(c p) -> p c", p=128),
                    in_=invt)
                invb = invp.tile([128, w], BF16, tag="invb",
                                 name=f"invb{s}")
                nc.sync.dma_start(
                    out=invb,
                    in_=scr_h[g][o0:o0 + w].partition_broadcast(128))
                apply_slab(s, invb)

            def apply_slab(s, invb):
                cols = slice(s * 1024, (s + 1) * 1024)
                for k in range(KB):
                    eng = nc.gpsimd if (s, k) in POOL_AP else nc.vector
                    eng.tensor_mul(out=xr[:, k, cols],
                                   in0=xr[:, k, cols], in1=invb)
                for k in range(KB):
                    junk = bitsp.tile([128, 1024], BF16, tag="junk",
                                      name=f"junk{s}_{k}")
                    nc.vector.tensor_scalar(
                        out=junk, in0=xr[:, k, cols],
                        scalar1=1.0, scalar2=0.0,
                        op0=ALU.mult, op1=ALU.add,
                        accum_out=pslab[:, k, s:s + 1],
                    )

            def cast_half(s):
                cols = slice(s * 1024, (s + 1) * 1024)
                nc.gpsimd.dma_start(out=xq[:, :, cols], in_=xr[:, :, cols])

            # ---- main loop tiles for group g, one half-sweep at a time so
            #      drains of half h only wait on slab 2g+h's apply ----
            vbits = {}

            def tiles(g):
                for half in range(2):
                    for j in range(MB):
                        t_idx = g * MB + j
                        is_v = t_idx in V_SET
                        if is_v and half == 0:
                            vbits[t_idx] = bitsp.tile(
                                [128, GW], I16, tag="bits",
                                name=f"bits{t_idx}")
                        ps = mmp.tile([128, 1024], F32, tag="mm",
                                      name=f"mm{t_idx}_{half}")
                        base = g * GW + half * 1024
                        for t in range(2):
                            csl = slice(base + t * 512, base + (t + 1) * 512)
                            if g in BF16_GROUPS:
                                for k in range(KB):
                                    nc.tensor.matmul(
                                        ps[:, t * 512:(t + 1) * 512],
                                        lhsT=rgbr[:, k, j * 128:(j + 1) * 128],
                                        rhs=xr[:, k, csl],
                                        start=(k == 0), stop=(k == KB - 1),
                                    )
                            else:
                                nc.tensor.matmul(
                                    ps[:, t * 512:(t + 1) * 512],
                                    lhsT=rgbq[:, :, j * 128:(j + 1) * 128],
                                    rhs=xq[:, :, csl],
                                    perf_mode=mybir.MatmulPerfMode.DoubleRow,
                                    start=True, stop=True,
                                )
                        if not is_v:
                            nc.scalar.activation(
                                out=ps, in_=ps, func=AF.Exp,
                                scale=scale_a[:, j:j + 1],
                                accum_out=accums[:, j * 8 + g * 2 + half:
                                                 j * 8 + g * 2 + half + 1],
                            )
                        else:
                            nc.vector.tensor_scalar(
                                out=vbits[t_idx][:, half * 1024:
                                                 (half + 1) * 1024],
                                in0=ps,
                                scalar1=scale_v[:, j:j + 1], scalar2=B16,
                                op0=ALU.mult, op1=ALU.add,
                            )
                        if half == 1:
                            if is_v:
                                bv = vbits.pop(t_idx).bitcast(BF16)
                                nc.vector.tensor_scalar(
                                    out=bv, in0=bv, scalar1=1.0, scalar2=0.0,
                                    op0=ALU.mult, op1=ALU.add,
                                    accum_out=accums[:, j * 8 + g * 2:
                                                     j * 8 + g * 2 + 1],
                                )
                            if g == NG - 1:
                                # row-block j done: fold its lse partial
                                nc.vector.reduce_sum(
                                    out=se[:, j:j + 1],
                                    in_=accums[:, j * 8:(j + 1) * 8],
                                    axis=mybir.AxisListType.X,
                                )
                                nc.scalar.activation(
                                    out=logs[:, j:j + 1],
                                    in_=se[:, j:j + 1], func=AF.Ln)

            # ---- declaration order tracks expected readiness; the sync
            #      DMA ring is hand-sequenced (loads and the tiny inv
            #      store/broadcast round-trips interleave) ----
            for k in range(KB):
                junk = bitsp.tile([128, 1024], BF16, tag="junk",
                                  name=f"junk0_{k}")
                nc.vector.tensor_scalar(
                    out=junk, in0=xr[:, k, 0:1024],
                    scalar1=1.0, scalar2=0.0,
                    op0=ALU.mult, op1=ALU.add,
                    accum_out=pslab[:, k, 0:1],
                )
            norm_slab(1)
            load_x(1, piece=1024)
            norm_slab(2)
            norm_slab(3)
            load_x(2, piece=1024)
            norm_slab(4)
            cast_half(4)
            norm_slab(5)
            cast_half(5)
            nc.sync.dma_start(out=rgbq,
                              in_=rgbq_h[:, :, :].rearrange("k c h -> c k h"))
            load_x(3, piece=1024)
            norm_slab(6)
            cast_half(6)
            norm_slab(7)
            cast_half(7)
            tiles(0)
            tiles(1)
            tiles(2)
            tiles(3)

            # ---- positives: pick this core's slab via one-hot sel, then
            #      q[m] = rgb[:, m] . P_d, partial = q * rs/(FQ*T) ----
            sel_b = small.tile([128, 8], BF16)
            nc.scalar.dma_start(out=sel_b, in_=sel_h[:].partition_broadcast(128))
            pkb = small.tile([128, KB], BF16)
            for k in range(KB):
                m8 = small.tile([128, 8], F32, tag=f"m8{k}", name=f"m8{k}")
                nc.vector.tensor_mul(out=m8, in0=pslab[:, k, :], in1=sel_b)
                pk = small.tile([128, 1], F32, tag=f"pk{k}", name=f"pk{k}")
                nc.vector.reduce_sum(out=pk, in_=m8, axis=mybir.AxisListType.X)
                nc.vector.tensor_copy(out=pkb[:, k:k + 1], in_=pk)
            pos_ps = smp.tile([128, MB], F32, tag="sm")
            for j in range(MB):
                for k in range(KB):
                    nc.tensor.matmul(
                        pos_ps[:, j:j + 1],
                        lhsT=rgbr[:, k, j * 128:(j + 1) * 128],
                        rhs=pkb[:, k:k + 1],
                        start=(k == 0), stop=(k == KB - 1),
                    )
            posq = small.tile([128, MB], F32)
            nc.vector.tensor_mul(out=posq, in0=pos_ps, in1=scale_a)
            nc.vector.reduce_sum(out=out_sb[:, 1:2], in_=posq,
                                 axis=mybir.AxisListType.X)

            # ---- logsumexp partials were folded per row-block in the g3
            #      tile stream; just reduce over j ----
            nc.vector.reduce_sum(out=out_sb[:, 0:1], in_=logs,
                                 axis=mybir.AxisListType.X)

            nc.scalar.dma_start(out=out_h[:, :], in_=out_sb)

    nc.finalize()
    return nc


def kernel(rgb_features, x_features):
    global LAST_RESULT
    rgb = np.ascontiguousarray(np.asarray(rgb_features, dtype=np.float32))
    x = np.ascontiguousarray(np.asarray(x_features, dtype=np.float32))
    assert rgb.shape == (B, C, 32, 32) and x.shape == (B, C, 32, 32)
    rgb = rgb.reshape(B, C, HW)
    x = x.reshape(B, C, HW)

    if "nc" not in _CACHE:
        _CACHE["nc"] = _build_nc()
    nc = _CACHE["nc"]

    # host-side input marshalling: SBUF-layout reorder + dtype casts.
    # x columns are permuted to n' = (n%8)*1024 + n//8 (n = b*1024 + h,
    # n%8 = h%8) so the positive columns of core d are slab d; row sums
    # of the logsumexp are invariant to the column order.
    xp = (x.reshape(B, KB, 128, 128, 8)
          .transpose(1, 2, 4, 0, 3)
          .reshape(KB, 128, N))
    xp = np.ascontiguousarray(xp).astype(ml_dtypes.bfloat16)
    in_maps = []
    for d in range(N_CORES):
        rgb_d = np.ascontiguousarray(rgb[d].reshape(KB, 128, HW))
        selv = np.zeros(8, dtype=ml_dtypes.bfloat16)
        selv[d] = 1.0
        in_maps.append({
            "xr": xp,
            "rgbr": rgb_d.astype(ml_dtypes.bfloat16),
            "rgbq": rgb_d.astype(ml_dtypes.float8_e4m3fn),
            "selv": selv,
        })

    try:
        res = run_bass_kernel_spmd(nc, in_maps, core_ids=list(range(N_CORES)))
    except ModuleNotFoundError:
        # BASS_TRACE set but this axon client lacks the NTFF profile hook
        # module; retry with tracing disabled.
        os.environ["BASS_NEVER_TRACE"] = "1"
        res = run_bass_kernel_spmd(nc, in_maps, core_ids=list(range(N_CORES)))
    LAST_RESULT = res

    L = 0.0
    P = 0.0
    for r in res.results:
        o = np.asarray(r["out"], dtype=np.float64)
        L += o[:, 0].sum()
        P += o[:, 1].sum()
    n_pos = float(N) * (N // 8)
    loss = -(P - (N // 8) * L) / (n_pos + 1e-8)
    return np.float32(loss)
